# revision 4
# baseline (speedup 1.0000x reference)
"""Local (sliding-window causal) attention on 8 Trainium2 NeuronCores.

Problem: B=1, S=4096, D=1024, H=16 heads (hd=64), WINDOW=256.
Sharding: tensor-parallel over heads -- 2 heads per core. Each core computes
q/k/v projections for its 2 heads, windowed softmax attention, and its
partial contribution o_c @ Wo_c. The host sums the 8 partials and adds the
bias terms.

Math notes:
 - score uses (q + bq) . (k + bk); the q.bk and bq.bk terms are constant per
   query row so they drop under softmax -> bk is dropped, bq folded into q.
 - v bias: o = p @ (v + bv) = p @ v + bv (softmax rows sum to 1), so the bv
   contribution to the output is the constant row bv @ Wo, added on host.
 - All matmuls run in float32r (tf32-like, full PE speed at N>=256),
   softmax statistics in f32.

Layouts on device (per core):
 - xT     [1024, 4096]   x transposed (host-provided), streamed in 512-col chunks
 - qT,kT  [128, S(+pad)]  head-dim on partitions (2 heads x 64), seq on free
 - v      [128, 34*128]  34 key blocks of [128 keys, 128 hd2]; first 2 blocks zero
 - oT     [128, 4096]    attention output transposed
 - y      [4096, 1024]   partial output (= oT.T @ Wo_c)

Sliding window: queries processed in super-blocks of 256 with a 512-key
padded window [sb*256-256, sb*256+256). Each 128-query half sees 384
contiguous keys of that window; bands masked additively before exp.
"""

import numpy as np

import concourse.bass as bass
import concourse.tile as tile
from concourse import bacc, mybir
from concourse.bass_utils import run_bass_kernel_spmd

# Problem constants (hardcoded per contract -- kernel.py must be self-contained)
S = 4096
D = 1024
H = 16
HD = 64
WINDOW = 256
N_CORES = 8
HPC = H // N_CORES          # heads per core = 2
DH = HPC * HD               # per-core head dims = 128
PAD = 256                   # zero left-padding of keys
SP = S + PAD                # padded key length = 4352
NEG = -1e9

F32 = mybir.dt.float32
F32R = mybir.dt.float32r

N_SB = S // 256             # 16 query super-blocks
N_QB = S // 128             # 32 query blocks
N_T = S // 512              # 8 projection seq chunks
KC = D // 128               # 8 contraction chunks


def _make_masks():
    """Additive masks [128, 384] for one 128-query half of a super-block.

    Query row qi (0..127) may see window-local columns jcol with
    qi+1 <= jcol <= qi+256 (same for both halves). For the first
    super-block, keys left of the sequence start are also masked:
    half 0 requires jcol >= 256, half 1 requires jcol >= 128.
    """
    qi = np.arange(128)[:, None]
    j = np.arange(384)[None, :]
    base = (j >= qi + 1) & (j <= qi + 256)
    m_g = np.where(base, 0.0, NEG).astype(np.float32)
    m0_h0 = np.where(base & (j >= 256), 0.0, NEG).astype(np.float32)
    m0_h1 = np.where(base & (j >= 128), 0.0, NEG).astype(np.float32)
    return m_g, m0_h0, m0_h1


def build_kernel():
    nc = bacc.Bacc()

    xT = nc.dram_tensor("xT", [D, S], F32, kind="ExternalInput")
    wq = nc.dram_tensor("wq", [D, DH], F32, kind="ExternalInput")
    wk = nc.dram_tensor("wk", [D, DH], F32, kind="ExternalInput")
    wv = nc.dram_tensor("wv", [D, DH], F32, kind="ExternalInput")
    bq = nc.dram_tensor("bq", [DH], F32, kind="ExternalInput")
    wo = nc.dram_tensor("wo", [DH, D], F32, kind="ExternalInput")
    y = nc.dram_tensor("y", [S, D], F32, kind="ExternalOutput")

    m_g, m0_h0, m0_h1 = _make_masks()
    mask_g_d = nc.inline_tensor(m_g, name="mask_g")
    mask0_d = [nc.inline_tensor(m0_h0, name="mask0_h0"),
               nc.inline_tensor(m0_h1, name="mask0_h1")]
    ident_d = nc.inline_tensor(np.eye(128, dtype=np.float32), name="ident")

    scale = 1.0 / float(np.sqrt(HD))

    with tile.TileContext(nc) as tc:
        with (
            tc.tile_pool(name="consts", bufs=1) as consts,
            tc.tile_pool(name="persist", bufs=1) as persist,
            tc.tile_pool(name="xstream", bufs=2) as xstream,
            tc.tile_pool(name="work", bufs=3) as work,
            tc.tile_pool(name="ppool", bufs=2) as ppool,
            tc.tile_pool(name="proj_ps", bufs=2, space="PSUM") as proj_ps,
            tc.tile_pool(name="attn_ps", bufs=3, space="PSUM") as attn_ps,
            tc.tile_pool(name="ot_ps", bufs=1, space="PSUM") as ot_ps,
            tc.tile_pool(name="y_ps", bufs=1, space="PSUM") as y_ps,
        ):
            # ---- constants to SBUF ----
            wq_t = consts.tile([128, KC * DH], F32R, name="wq_t")
            wk_t = consts.tile([128, KC * DH], F32R, name="wk_t")
            wv_t = consts.tile([128, KC * DH], F32R, name="wv_t")
            for (t, d) in ((wq_t, wq), (wk_t, wk), (wv_t, wv)):
                d3 = d.ap().rearrange("(c p) m -> p c m", p=128)
                for c in range(KC):
                    nc.sync.dma_start(t[:, c * DH:(c + 1) * DH], d3[:, c].bitcast(F32R))
            wo_t = consts.tile([DH, D], F32R, name="wo_t")
            nc.sync.dma_start(wo_t, wo.ap().bitcast(F32R))

            mask_g = consts.tile([128, 384], F32, name="mask_g")
            nc.sync.dma_start(mask_g, mask_g_d.ap())
            mask0 = []
            for u in range(2):
                mt = consts.tile([128, 384], F32, name=f"mask0_{u}", tag=f"mask0_{u}")
                nc.sync.dma_start(mt, mask0_d[u].ap())
                mask0.append(mt)
            ident = consts.tile([128, 128], F32R, name="ident")
            nc.sync.dma_start(ident, ident_d.ap().bitcast(F32R))

            bq_t = consts.tile([DH, 1], F32, name="bq_t")
            nc.sync.dma_start(bq_t, bq.ap().rearrange("(p o) -> p o", o=1))
            bqs = consts.tile([DH, 1], F32, name="bqs")
            nc.vector.tensor_scalar_mul(bqs, bq_t, scale)

            # ---- persistent activations ----
            qT = persist.tile([128, S], F32R, name="qT")
            kT = persist.tile([128, SP], F32R, name="kT")
            vv = persist.tile([128, (SP // 128) * 128], F32R, name="vv")
            oT = persist.tile([128, S], F32R, name="oT")
            nc.vector.memset(kT[:, 0:PAD].bitcast(F32), 0.0)
            nc.vector.memset(vv[:, 0:PAD].bitcast(F32), 0.0)

            # ---- projections ----
            for t in range(N_T):
                sl = slice(t * 512, (t + 1) * 512)
                xt = xstream.tile([128, KC, 512], F32R, name="xt")
                for c in range(KC):
                    nc.sync.dma_start(
                        xt[:, c], xT.ap()[c * 128:(c + 1) * 128, sl].bitcast(F32R))

                qps = proj_ps.tile([128, 512], F32, name="pps", tag="pps")
                for c in range(KC):
                    nc.tensor.matmul(qps, wq_t[:, c * DH:(c + 1) * DH], xt[:, c],
                                     start=(c == 0), stop=(c == KC - 1))
                nc.scalar.activation(qT[:, sl], qps,
                                     mybir.ActivationFunctionType.Identity,
                                     bias=bqs, scale=scale)

                kps = proj_ps.tile([128, 512], F32, name="kps", tag="pps")
                for c in range(KC):
                    nc.tensor.matmul(kps, wk_t[:, c * DH:(c + 1) * DH], xt[:, c],
                                     start=(c == 0), stop=(c == KC - 1))
                nc.scalar.copy(kT[:, PAD + t * 512:PAD + (t + 1) * 512], kps)

                vps = proj_ps.tile([128, 512], F32, name="vps", tag="pps")
                for c in range(KC):
                    nc.tensor.matmul(vps, wv_t[:, c * DH:(c + 1) * DH], xt[:, c],
                                     start=(c == 0), stop=(c == KC - 1))
                vt = work.tile([128, 512], F32R, name="vt", tag="vt")
                nc.scalar.copy(vt, vps)
                # transpose [hd2, seq] -> [seq, hd2] blocks into vv
                for b in range(4):
                    tb = proj_ps.tile([128, 128], F32R, name="tb", tag="pps")
                    nc.tensor.transpose(tb, vt[:, b * 128:(b + 1) * 128], ident)
                    blk = 2 + t * 4 + b
                    nc.scalar.copy(vv[:, blk * 128:(blk + 1) * 128], tb)

            # ---- attention ----
            for sb in range(N_SB):
                pT = []
                for h in range(2):
                    pt = ppool.tile([128, 4, 2, 128], F32R, name=f"pT{h}",
                                    tag=f"pT{h}")
                    nc.gpsimd.memset(pt[:, 3, 0].bitcast(F32), 0.0)
                    nc.gpsimd.memset(pt[:, 0, 1].bitcast(F32), 0.0)
                    pT.append(pt)
                for u in range(2):
                    qb = sb * 2 + u
                    qsl = slice(qb * 128, (qb + 1) * 128)
                    wsl = slice(sb * 256 + u * 128, sb * 256 + u * 128 + 384)
                    mask = mask0[u] if sb == 0 else mask_g
                    for h in range(2):
                        hsl = slice(h * 64, (h + 1) * 64)
                        sps = attn_ps.tile([128, 384], F32, name="sps", tag="aps")
                        nc.tensor.matmul(sps, qT[hsl, qsl], kT[hsl, wsl],
                                         start=True, stop=True)
                        sm = work.tile([128, 384], F32, name="sm", tag="sm")
                        nc.vector.scalar_tensor_tensor(
                            sm, sps, 1.0, mask,
                            op0=mybir.AluOpType.mult, op1=mybir.AluOpType.add)
                        p = work.tile([128, 384], F32R, name="p", tag="p")
                        rs = work.tile([128, 1], F32, name="rs", tag="rs")
                        nc.scalar.activation(p, sm,
                                             mybir.ActivationFunctionType.Exp,
                                             accum_out=rs)
                        rc = work.tile([128, 1], F32, name="rc", tag="rc")
                        nc.vector.reciprocal(rc, rs)
                        pn = work.tile([128, 384], F32R, name="pn", tag="pn")
                        nc.vector.tensor_scalar_mul(pn, p, rc)
                        tps = attn_ps.tile([128, 384], F32R, name="tps", tag="aps")
                        for kb3 in range(3):
                            nc.tensor.transpose(tps[:, kb3 * 128:(kb3 + 1) * 128],
                                                pn[:, kb3 * 128:(kb3 + 1) * 128],
                                                ident)
                        # window-local key blocks u..u+2, half u
                        nc.vector.tensor_copy(pT[h][:, u:u + 3, u], tps)

                ot = ot_ps.tile([64, 512], F32, name="ot")
                for h in range(2):
                    for kb in range(4):
                        blk = sb * 2 + kb
                        vsl = vv[:, blk * 128 + h * 64: blk * 128 + h * 64 + 64]
                        nc.tensor.matmul(ot[:, h * 256:h * 256 + 256], vsl,
                                         pT[h][:, kb],
                                         start=(kb == 0), stop=(kb == 3))
                ssl = slice(sb * 256, (sb + 1) * 256)
                nc.scalar.copy(oT[0:64, ssl], ot[:, 0:256])
                nc.scalar.copy(oT[64:128, ssl], ot[:, 256:512])

            # ---- output projection (partial: this core's heads only) ----
            for qb in range(N_QB):
                yps = y_ps.tile([128, 1024], F32, name="yps")
                for nch in range(2):
                    nc.tensor.matmul(yps[:, nch * 512:(nch + 1) * 512],
                                     oT[:, qb * 128:(qb + 1) * 128],
                                     wo_t[:, nch * 512:(nch + 1) * 512],
                                     start=True, stop=True)
                ysb = work.tile([128, 1024], F32, name="ysb", tag="ysb")
                nc.vector.tensor_copy(ysb, yps)
                nc.sync.dma_start(y.ap()[qb * 128:(qb + 1) * 128, :], ysb)

    if not nc.is_finalized():
        nc.finalize()
    return nc


_NC_CACHE = None


def kernel(x, Wq, bq, Wk, bk, Wv, bv, Wo, bo, **_kw):
    global _NC_CACHE
    x = np.asarray(x, dtype=np.float32)
    Wq = np.asarray(Wq, dtype=np.float32)
    Wk = np.asarray(Wk, dtype=np.float32)
    Wv = np.asarray(Wv, dtype=np.float32)
    Wo = np.asarray(Wo, dtype=np.float32)
    bq = np.asarray(bq, dtype=np.float32)
    bv = np.asarray(bv, dtype=np.float32)
    bo = np.asarray(bo, dtype=np.float32)

    B = x.shape[0]
    assert x.shape == (B, S, D) and B == 1

    xT = np.ascontiguousarray(x[0].T)

    in_maps = []
    for c in range(N_CORES):
        csl = slice(c * DH, (c + 1) * DH)
        in_maps.append({
            "xT": xT,
            "wq": np.ascontiguousarray(Wq[:, csl]),
            "wk": np.ascontiguousarray(Wk[:, csl]),
            "wv": np.ascontiguousarray(Wv[:, csl]),
            "bq": np.ascontiguousarray(bq[csl]),
            "wo": np.ascontiguousarray(Wo[csl, :]),
        })

    if _NC_CACHE is None:
        _NC_CACHE = build_kernel()
    res = run_bass_kernel_spmd(_NC_CACHE, in_maps, core_ids=list(range(N_CORES)))

    out = np.zeros((S, D), dtype=np.float32)
    for c in range(N_CORES):
        out += res.results[c]["y"]
    # host-side bias terms: bo plus the bv @ Wo constant row (see header)
    out += (bv @ Wo + bo)[None, :]
    return out.reshape(1, S, D)
